# revision 13
# baseline (speedup 1.0000x reference)
"""GQA attention (B=2,T=2048,D=4096, 32Q/8KV heads, RoPE, causal) on 8 TRN2 cores.

Sharding: core c = (batch b = c//4, head-group g = c%4). Each core computes its
batch's attention for 8 query heads (global 8g..8g+8) + their 2 aligned KV heads
(global 2g..2g+2), and applies its slice of wo -> a partial [T, D] output.
Host sums the 4 head-group partials per batch. No collectives.

v3: one interleaved program so the Tile scheduler keeps the PE saturated:
  - Attention units (h, b) woven between projection taus (2 per tau >= 4);
    output-projection taus alternate with the b=3 units at the tail. ScalarE
    exp hides under projection/wo matmuls instead of serializing.
  - Lag-2 software pipeline inside each unit: PV + rowsum-accumulate trail
    S/exp by 2 strips so PV never blocks the in-order PE queue at runtime.
  - Softmax denominator: P^T strips accumulated on DVE (bf16), one
    ones-matmul per unit (vs per strip: saves ~139k PE cycles).
  - Causal triangle applied as a post-exp 0/1 multiply on DVE (diag tile
    only) -- no PE mask matmuls, no DVE-in-the-S->exp chain.
  - wqkv stored oc-major (contiguous slices): kv slice lands first and the
    PE starts ~6us in instead of ~32us (tau oc order kv-first).
"""
import numpy as np
import ml_dtypes

import concourse.bass as bass
import concourse.mybir as mybir
from concourse import bacc, tile
from concourse.bass_utils import run_bass_kernel_spmd

bf16 = mybir.dt.bfloat16
f32 = mybir.dt.float32
BF = ml_dtypes.bfloat16

B, T, D = 2, 2048, 4096
NQ, NKV, HD = 32, 8, 128
HQ, HKV = 8, 2            # per-core heads
NT = T // 128             # 16 token tiles
NC = D // 128             # 32 contraction chunks
NB = NT // 4              # 4 tq blocks of 512
SCALE = 1.0 / np.sqrt(HD)
LAG = 2                   # strips PV/acc trail behind S/exp


def _build_nc():
    nc = bacc.Bacc(None, target_bir_lowering=False)
    xt_ext = nc.declare_dram_parameter("xt", [NT, 128, NC, 128], bf16, isOutput=False)
    wqkv_ext = nc.declare_dram_parameter("wqkv", [3, 128, NC, 512], bf16, isOutput=False)
    wo_ext = nc.declare_dram_parameter("wo", [128, HQ, D], bf16, isOutput=False)
    rope_ext = nc.declare_dram_parameter("rope", [128, NT, 1024], bf16, isOutput=False)
    m01_ext = nc.declare_dram_parameter("mask01", [128, 128], bf16, isOutput=False)
    id_ext = nc.declare_dram_parameter("ident", [128, 128], bf16, isOutput=False)
    out_ext = nc.declare_dram_parameter("out", [T, D], f32, isOutput=True)

    with tile.TileContext(nc) as tc:
        with (
            tc.tile_pool(name="persist", bufs=1) as persist,
            tc.tile_pool(name="qtbp", bufs=3) as qtbp,
            tc.tile_pool(name="aotp", bufs=3) as aotp,
            tc.tile_pool(name="ptsp", bufs=5) as ptsp,
            tc.tile_pool(name="accp", bufs=1) as accp,
            tc.tile_pool(name="recp", bufs=1) as recp,
            tc.tile_pool(name="spsB", bufs=3, space="PSUM") as spsB,
            tc.tile_pool(name="otB", bufs=2, space="PSUM") as otB,
        ):
            ktb = [persist.tile([128, HKV, 512], bf16, tag=f"kt{j}", name=f"kt{j}") for j in range(NB)]
            vbb = [persist.tile([128, 4 * 256], bf16, tag=f"vb{j}", name=f"vb{j}") for j in range(NB)]
            ident = persist.tile([128, 128], bf16, tag="ident")
            m01 = persist.tile([128, 128], bf16, tag="m01")
            ones = persist.tile([128, 128], bf16, tag="ones")
            nc.sync.dma_start(ident[:], id_ext[:])
            nc.gpsimd.dma_start(m01[:], m01_ext[:])
            nc.vector.memset(ones[:], 1.0)

            qtb = {}   # b -> [128, HQ, 512] tile
            aotb = {}  # b -> [128, HQ, 512] tile

            def emit_B_unit(h, b):
                """Attention for query head h over tq block b (512 tokens)."""
                kvh = h // 4
                nstrip = 4 * (b + 1)
                ot = otB.tile([128, 512], f32, tag="otrs", name=f"ot{h}_{b}")
                acc = accp.tile([128, 512], bf16, tag="acc")
                pend = []

                def flush_one():
                    t, lo, pts = pend.pop(0)
                    nc.tensor.matmul(
                        ot[:, lo:512],
                        vbb[t // 4][:, (t % 4) * 256 + kvh * 128:
                                    (t % 4) * 256 + (kvh + 1) * 128],
                        pts[:, lo:512],
                        start=(t == 0), stop=(t == nstrip - 1))
                    if t == 0:
                        nc.vector.tensor_copy(acc[:], pts[:, 0:512])
                    else:
                        nc.vector.tensor_add(acc[:, lo:512], acc[:, lo:512],
                                             pts[:, lo:512])

                for t in range(nstrip):
                    r = t - 4 * b
                    lo = 128 * r if r > 0 else 0
                    s_ps = spsB.tile([128, 512], f32, tag="s")
                    nc.tensor.matmul(
                        s_ps[:, lo:512],
                        ktb[t // 4][:, kvh, (t % 4) * 128:(t % 4 + 1) * 128],
                        qtb[b][:, h, lo:512],
                        start=True, stop=True)
                    pts = ptsp.tile([128, 512], bf16, tag="pts")
                    nc.scalar.activation(
                        pts[:, lo:512], s_ps[:, lo:512],
                        mybir.ActivationFunctionType.Exp,
                        bias=0.0, scale=SCALE)
                    if r >= 0:  # zero the strictly-upper triangle of diag tile
                        nc.gpsimd.tensor_mul(
                            pts[:, 128 * r:128 * (r + 1)],
                            pts[:, 128 * r:128 * (r + 1)], m01[:])
                    pend.append((t, lo, pts))
                    if len(pend) > LAG:
                        flush_one()
                while pend:
                    flush_one()
                rs = otB.tile([128, 512], f32, tag="otrs", name=f"rs{h}_{b}")
                nc.tensor.matmul(rs[:], ones[:], acc[:], start=True, stop=True)
                recip = recp.tile([128, 512], f32, tag="recip")
                nc.vector.reciprocal_approx_fast(out=recip[:], in_=rs[:])
                nc.vector.tensor_mul(aotb[b][:, h, :], ot[:], recip[:])

            # ------------- Phase A (+ woven attention units) -----------------
            unit_queue = []
            with (
                tc.tile_pool(name="wqkvp", bufs=1) as wqkvp,
                tc.tile_pool(name="xtp", bufs=2) as xtp,
                tc.tile_pool(name="ropep", bufs=2) as ropep,
                tc.tile_pool(name="rotp", bufs=2) as rotp,
                tc.tile_pool(name="psA", bufs=2, space="PSUM") as psA,
                tc.tile_pool(name="ptA", bufs=1, space="PSUM") as ptA,
            ):
                wqkv = wqkvp.tile([128, 3, NC, 512], bf16, tag="wqkv")
                # oc-major contiguous pieces (4 chunks each), kv (oc=2) first.
                # xt/rope DMAs are emitted first on their queues each tau; the
                # first-needed kv piece rides the otherwise-idle scalar queue.
                xts0 = xtp.tile([128, NC, 128], bf16, tag="xt", name="xts0")
                nc.sync.dma_start(xts0[:], xt_ext[0])
                rope0 = ropep.tile([128, 1024], bf16, tag="rope", name="rope0")
                nc.gpsimd.dma_start(rope0[:], rope_ext[:, 0, :])
                qs = [nc.sync, nc.gpsimd, nc.scalar]
                qi = 0
                for oc in (2, 0, 1):
                    for g in range(4):
                        qs[qi % 3].dma_start(
                            wqkv[:, oc, 8 * g:8 * (g + 1), :],
                            wqkv_ext[oc, :, 8 * g:8 * (g + 1), :])
                        qi += 1

                def emit_A_tau(tau, ocs):
                    if tau == 0:
                        xts, rope = xts0, rope0
                    else:
                        xts = xtp.tile([128, NC, 128], bf16, tag="xt")
                        nc.sync.dma_start(xts[:], xt_ext[tau])
                        rope = ropep.tile([128, 1024], bf16, tag="rope")
                        nc.gpsimd.dma_start(rope[:], rope_ext[:, tau, :])
                    cc, ss = rope[:, 0:512], rope[:, 512:1024]
                    if 0 in ocs and tau % 4 == 0:
                        qtb[tau // 4] = qtbp.tile([128, HQ, 512], bf16,
                                                  tag="qtb", name=f"qtb{tau // 4}")
                    for oc in ocs:
                        ps = psA.tile([128, 512], f32, tag="proj")
                        for c in range(NC):
                            nc.tensor.matmul(
                                ps[:], xts[:, c, :], wqkv[:, oc, c, :],
                                start=(c == 0), stop=(c == NC - 1))
                        rt = rotp.tile([128, 1024], bf16, tag="rot")
                        rot, tmp = rt[:, 0:512], rt[:, 512:1024]
                        if oc < 2:  # 4 q heads
                            _rope(nc, ps[:], cc, ss, rot, tmp)
                            pt = ptA.tile([128, 512], bf16, tag="ptA")
                            for j in range(4):
                                nc.tensor.transpose(
                                    pt[:, j * 128:(j + 1) * 128],
                                    rot[:, j * 128:(j + 1) * 128], ident[:])
                            nc.vector.tensor_copy(
                                qtb[tau // 4][:, oc * 4:(oc + 1) * 4,
                                              (tau % 4) * 128:(tau % 4 + 1) * 128],
                                pt[:].rearrange("p (h t) -> p h t", h=4))
                        else:  # 2 k heads + 2 v heads
                            _rope(nc, ps[:, 0:256], cc[:, 0:256], ss[:, 0:256],
                                  rot[:, 0:256], tmp[:, 0:256])
                            pt = ptA.tile([128, 512], bf16, tag="ptA")
                            for j in range(2):
                                nc.tensor.transpose(
                                    pt[:, j * 128:(j + 1) * 128],
                                    rot[:, j * 128:(j + 1) * 128], ident[:])
                            nc.vector.tensor_copy(
                                ktb[tau // 4][:, :, (tau % 4) * 128:(tau % 4 + 1) * 128],
                                pt[:, 0:256].rearrange("p (h t) -> p h t", h=2))
                            nc.vector.tensor_copy(
                                vbb[tau // 4][:, (tau % 4) * 256:(tau % 4 + 1) * 256],
                                ps[:, 256:512])
                    if 1 in ocs and tau % 4 == 3:
                        b = tau // 4
                        aotb[b] = aotp.tile([128, HQ, 512], bf16,
                                            tag="aot", name=f"aot{b}")
                        unit_queue.extend((h, b) for h in range(HQ))

                for tau in range(NT):
                    emit_A_tau(tau, (2, 0, 1))
                    if tau >= 4:
                        for _ in range(2):
                            emit_B_unit(*unit_queue.pop(0))

            # ------------- Tail: b=3 units alternating with out-proj taus ----
            with (
                tc.tile_pool(name="wop", bufs=1) as wop,
                tc.tile_pool(name="outp", bufs=4) as outp,
                tc.tile_pool(name="psC", bufs=3, space="PSUM") as psC,
            ):
                wo = wop.tile([128, HQ, D], bf16, tag="wo")
                for nck in range(8):
                    (nc.sync if nck % 2 == 0 else nc.gpsimd).dma_start(
                        wo[:, :, nck * 512:(nck + 1) * 512],
                        wo_ext[:, :, nck * 512:(nck + 1) * 512])

                def emit_C_tau(tau):
                    b = tau // 4
                    for nck in range(8):
                        o_ps = psC.tile([128, 512], f32, tag="o")
                        for h in range(HQ):
                            nc.tensor.matmul(
                                o_ps[:],
                                aotb[b][:, h, (tau % 4) * 128:(tau % 4 + 1) * 128],
                                wo[:, h, nck * 512:(nck + 1) * 512],
                                start=(h == 0), stop=(h == HQ - 1))
                        ost = outp.tile([128, 512], f32, tag="ost")
                        nc.vector.tensor_copy(ost[:], o_ps[:])
                        (nc.sync if nck % 2 == 0 else nc.gpsimd).dma_start(
                            out_ext[tau * 128:(tau + 1) * 128,
                                    nck * 512:(nck + 1) * 512],
                            ost[:])

                # alternate unit / out-proj tau, then the remaining taus
                for i in range(8):
                    emit_B_unit(*unit_queue.pop(0))
                    emit_C_tau(i)
                for tau in range(8, NT):
                    emit_C_tau(tau)

    nc.compile()
    return nc


def _rope(nc, ps, cc, ss, rot, tmp):
    """rot = ps*cc + pairswap(ps)*ss   (pairs are consecutive elements)."""
    swap = ps.rearrange("p (i two) -> p i two", two=2)[:, :, ::-1]
    nc.vector.tensor_mul(tmp.rearrange("p (i two) -> p i two", two=2), swap,
                         ss.rearrange("p (i two) -> p i two", two=2))
    nc.vector.tensor_mul(rot, ps, cc)
    nc.vector.tensor_add(rot, rot, tmp)


_NC_CACHE = None


def _get_nc():
    global _NC_CACHE
    if _NC_CACHE is None:
        _NC_CACHE = _build_nc()
    return _NC_CACHE


def _rope_tables():
    i = np.arange(HD // 2, dtype=np.float64)
    theta = np.power(10000.0, -2.0 * i / HD)
    ang = np.outer(np.arange(T, dtype=np.float64), theta)    # [T, 64]
    cos, sin = np.cos(ang), np.sin(ang)
    cc128 = np.repeat(cos, 2, axis=1)                        # [T, 128]
    ss128 = np.stack([-sin, sin], axis=-1).reshape(T, HD)    # [T, 128]
    cc = np.tile(cc128, (1, 4))                              # [T, 512]
    ss = np.tile(ss128, (1, 4))
    ropeccss = np.concatenate([cc, ss], axis=1)              # [T, 1024]
    return np.ascontiguousarray(
        ropeccss.reshape(NT, 128, 1024).transpose(1, 0, 2)).astype(BF)


def _mask01():
    # multiplicative causal mask for the diagonal 128x128 tile of a P^T strip:
    # partition tk, free tq_local; keep iff tq_local >= tk
    i = np.arange(128)
    return (i[None, :] >= i[:, None]).astype(BF)


def _prep_core_inputs(x, wq, wk, wv, wo):
    rope = _rope_tables()
    m01 = _mask01()
    ident = np.eye(128).astype(BF)
    in_maps = []
    for c in range(8):
        b, g = c // 4, c % 4
        xb = np.asarray(x[b], dtype=np.float32)
        xt = np.ascontiguousarray(
            xb.reshape(NT, 128, NC, 128).transpose(0, 3, 2, 1)).astype(BF)
        wq_g = wq[:, g * 8 * HD:(g + 1) * 8 * HD]
        wk_g = wk[:, g * 2 * HD:(g + 1) * 2 * HD]
        wv_g = wv[:, g * 2 * HD:(g + 1) * 2 * HD]
        W = np.concatenate([wq_g, wk_g, wv_g], axis=1)       # [D, 1536]
        wqkv3 = np.stack([
            np.ascontiguousarray(
                W[:, oc * 512:(oc + 1) * 512].reshape(NC, 128, 512)
                .transpose(1, 0, 2))
            for oc in range(3)], axis=0).astype(BF)          # [3, 128, NC, 512]
        wo_g = wo[g * 8 * HD:(g + 1) * 8 * HD, :]            # [1024, D]
        wo_t = np.ascontiguousarray(
            wo_g.reshape(HQ, 128, D).transpose(1, 0, 2)).astype(BF)
        in_maps.append({
            "xt": xt, "wqkv": wqkv3, "wo": wo_t,
            "rope": rope, "mask01": m01, "ident": ident,
        })
    return in_maps


def _run(inputs, trace=False, trace_kwargs=None):
    x = np.asarray(inputs["x"], dtype=np.float32)
    wq = np.asarray(inputs["wq"], dtype=np.float32)
    wk = np.asarray(inputs["wk"], dtype=np.float32)
    wv = np.asarray(inputs["wv"], dtype=np.float32)
    wo = np.asarray(inputs["wo"], dtype=np.float32)
    nc = _get_nc()
    in_maps = _prep_core_inputs(x, wq, wk, wv, wo)
    res = run_bass_kernel_spmd(nc, in_maps, core_ids=list(range(8)),
                               trace=trace, **(trace_kwargs or {}))
    out = np.zeros((B, T, D), dtype=np.float32)
    for c in range(8):
        out[c // 4] += res.results[c]["out"]
    return out, res


def kernel(**inputs):
    out, _ = _run(inputs)
    return out


# revision 14
# speedup vs baseline: 1.1223x; 1.1223x over previous
"""GQA attention (B=2,T=2048,D=4096, 32Q/8KV heads, RoPE, causal) on 8 TRN2 cores.

Sharding: core c = (batch b = c//4, head-group g = c%4). Each core computes its
batch's attention for 8 query heads (global 8g..8g+8) + their 2 aligned KV heads
(global 2g..2g+2), and applies its slice of wo -> a partial [T, D] output.
Host sums the 4 head-group partials per batch. No collectives.

v3: one interleaved program so the Tile scheduler keeps the PE saturated:
  - Attention units (h, b) woven between projection taus (2 per tau >= 4);
    output-projection taus alternate with the b=3 units at the tail. ScalarE
    exp hides under projection/wo matmuls instead of serializing.
  - Lag-2 software pipeline inside each unit: PV + rowsum-accumulate trail
    S/exp by 2 strips so PV never blocks the in-order PE queue at runtime.
  - Softmax denominator: P^T strips accumulated on DVE (bf16), one
    ones-matmul per unit (vs per strip: saves ~139k PE cycles).
  - Causal triangle applied as a post-exp 0/1 multiply on DVE (diag tile
    only) -- no PE mask matmuls, no DVE-in-the-S->exp chain.
  - wqkv stored oc-major (contiguous slices): kv slice lands first and the
    PE starts ~6us in instead of ~32us (tau oc order kv-first).
"""
import numpy as np
import ml_dtypes

import concourse.bass as bass
import concourse.mybir as mybir
from concourse import bacc, tile
from concourse.bass_utils import run_bass_kernel_spmd

bf16 = mybir.dt.bfloat16
f32 = mybir.dt.float32
BF = ml_dtypes.bfloat16

B, T, D = 2, 2048, 4096
NQ, NKV, HD = 32, 8, 128
HQ, HKV = 8, 2            # per-core heads
NT = T // 128             # 16 token tiles
NC = D // 128             # 32 contraction chunks
NB = NT // 4              # 4 tq blocks of 512
SCALE = 1.0 / np.sqrt(HD)
LAG = 2                   # strips PV/acc trail behind S/exp


def _build_nc():
    nc = bacc.Bacc(None, target_bir_lowering=False)
    xt_ext = nc.declare_dram_parameter("xt", [NT, 128, NC, 128], bf16, isOutput=False)
    wqkv_ext = nc.declare_dram_parameter("wqkv", [3, 128, NC, 512], bf16, isOutput=False)
    wo_ext = nc.declare_dram_parameter("wo", [128, HQ, D], bf16, isOutput=False)
    rope_ext = nc.declare_dram_parameter("rope", [128, NT, 1024], bf16, isOutput=False)
    m01_ext = nc.declare_dram_parameter("mask01", [128, 128], bf16, isOutput=False)
    id_ext = nc.declare_dram_parameter("ident", [128, 128], bf16, isOutput=False)
    out_ext = nc.declare_dram_parameter("out", [T, D], f32, isOutput=True)

    with tile.TileContext(nc) as tc:
        with (
            tc.tile_pool(name="persist", bufs=1) as persist,
            tc.tile_pool(name="qtbp", bufs=3) as qtbp,
            tc.tile_pool(name="aotp", bufs=3) as aotp,
            tc.tile_pool(name="ptsp", bufs=5) as ptsp,
            tc.tile_pool(name="accp", bufs=1) as accp,
            tc.tile_pool(name="recp", bufs=1) as recp,
            tc.tile_pool(name="spsB", bufs=3, space="PSUM") as spsB,
            tc.tile_pool(name="otB", bufs=2, space="PSUM") as otB,
        ):
            ktb = [persist.tile([128, HKV, 512], bf16, tag=f"kt{j}", name=f"kt{j}") for j in range(NB)]
            vbb = [persist.tile([128, 4 * 256], bf16, tag=f"vb{j}", name=f"vb{j}") for j in range(NB)]
            ident = persist.tile([128, 128], bf16, tag="ident")
            m01 = persist.tile([128, 128], bf16, tag="m01")
            ones = persist.tile([128, 128], bf16, tag="ones")
            nc.sync.dma_start(ident[:], id_ext[:])
            nc.gpsimd.dma_start(m01[:], m01_ext[:])
            nc.vector.memset(ones[:], 1.0)

            qtb = {}   # b -> [128, HQ, 512] tile
            aotb = {}  # b -> [128, HQ, 512] tile

            def emit_B_unit(h, b):
                """Attention for query head h over tq block b (512 tokens)."""
                kvh = h // 4
                nstrip = 4 * (b + 1)
                ot = otB.tile([128, 512], f32, tag="otrs", name=f"ot{h}_{b}")
                acc = accp.tile([128, 512], bf16, tag="acc")
                pend = []

                def flush_one():
                    t, lo, pts = pend.pop(0)
                    nc.tensor.matmul(
                        ot[:, lo:512],
                        vbb[t // 4][:, (t % 4) * 256 + kvh * 128:
                                    (t % 4) * 256 + (kvh + 1) * 128],
                        pts[:, lo:512],
                        start=(t == 0), stop=(t == nstrip - 1))
                    if t == 0:
                        nc.vector.tensor_copy(acc[:], pts[:, 0:512])
                    else:
                        nc.vector.tensor_add(acc[:, lo:512], acc[:, lo:512],
                                             pts[:, lo:512])

                for t in range(nstrip):
                    r = t - 4 * b
                    lo = 128 * r if r > 0 else 0
                    s_ps = spsB.tile([128, 512], f32, tag="s")
                    nc.tensor.matmul(
                        s_ps[:, lo:512],
                        ktb[t // 4][:, kvh, (t % 4) * 128:(t % 4 + 1) * 128],
                        qtb[b][:, h, lo:512],
                        start=True, stop=True)
                    pts = ptsp.tile([128, 512], bf16, tag="pts")
                    nc.scalar.activation(
                        pts[:, lo:512], s_ps[:, lo:512],
                        mybir.ActivationFunctionType.Exp,
                        bias=0.0, scale=SCALE)
                    if r >= 0:  # zero the strictly-upper triangle of diag tile
                        nc.vector.tensor_mul(
                            pts[:, 128 * r:128 * (r + 1)],
                            pts[:, 128 * r:128 * (r + 1)], m01[:])
                    pend.append((t, lo, pts))
                    if len(pend) > LAG:
                        flush_one()
                while pend:
                    flush_one()
                rs = otB.tile([128, 512], f32, tag="otrs", name=f"rs{h}_{b}")
                nc.tensor.matmul(rs[:], ones[:], acc[:], start=True, stop=True)
                recip = recp.tile([128, 512], f32, tag="recip")
                nc.vector.reciprocal_approx_fast(out=recip[:], in_=rs[:])
                nc.vector.tensor_mul(aotb[b][:, h, :], ot[:], recip[:])

            # ------------- Phase A (+ woven attention units) -----------------
            unit_queue = []
            with (
                tc.tile_pool(name="wqkvp", bufs=1) as wqkvp,
                tc.tile_pool(name="xtp", bufs=2) as xtp,
                tc.tile_pool(name="ropep", bufs=2) as ropep,
                tc.tile_pool(name="rotp", bufs=2) as rotp,
                tc.tile_pool(name="psA", bufs=2, space="PSUM") as psA,
                tc.tile_pool(name="ptA", bufs=1, space="PSUM") as ptA,
            ):
                wqkv = wqkvp.tile([128, 3, NC, 512], bf16, tag="wqkv")
                # oc-major contiguous pieces (4 chunks each), kv (oc=2) first.
                # xt/rope DMAs are emitted first on their queues each tau; the
                # first-needed kv piece rides the otherwise-idle scalar queue.
                qs = [nc.sync, nc.gpsimd, nc.scalar]
                qi = 0
                for oc in (2, 0, 1):
                    for g in range(4):
                        qs[qi % 3].dma_start(
                            wqkv[:, oc, 8 * g:8 * (g + 1), :],
                            wqkv_ext[oc, :, 8 * g:8 * (g + 1), :])
                        qi += 1

                def emit_A_tau(tau, ocs):
                    xts = xtp.tile([128, NC, 128], bf16, tag="xt")
                    nc.sync.dma_start(xts[:], xt_ext[tau])
                    rope = ropep.tile([128, 1024], bf16, tag="rope")
                    nc.gpsimd.dma_start(rope[:], rope_ext[:, tau, :])
                    cc, ss = rope[:, 0:512], rope[:, 512:1024]
                    if 0 in ocs and tau % 4 == 0:
                        qtb[tau // 4] = qtbp.tile([128, HQ, 512], bf16,
                                                  tag="qtb", name=f"qtb{tau // 4}")
                    for oc in ocs:
                        ps = psA.tile([128, 512], f32, tag="proj")
                        for c in range(NC):
                            nc.tensor.matmul(
                                ps[:], xts[:, c, :], wqkv[:, oc, c, :],
                                start=(c == 0), stop=(c == NC - 1))
                        rt = rotp.tile([128, 1024], bf16, tag="rot")
                        rot, tmp = rt[:, 0:512], rt[:, 512:1024]
                        if oc < 2:  # 4 q heads
                            _rope(nc, ps[:], cc, ss, rot, tmp)
                            pt = ptA.tile([128, 512], bf16, tag="ptA")
                            for j in range(4):
                                nc.tensor.transpose(
                                    pt[:, j * 128:(j + 1) * 128],
                                    rot[:, j * 128:(j + 1) * 128], ident[:])
                            nc.vector.tensor_copy(
                                qtb[tau // 4][:, oc * 4:(oc + 1) * 4,
                                              (tau % 4) * 128:(tau % 4 + 1) * 128],
                                pt[:].rearrange("p (h t) -> p h t", h=4))
                        else:  # 2 k heads + 2 v heads
                            _rope(nc, ps[:, 0:256], cc[:, 0:256], ss[:, 0:256],
                                  rot[:, 0:256], tmp[:, 0:256])
                            pt = ptA.tile([128, 512], bf16, tag="ptA")
                            for j in range(2):
                                nc.tensor.transpose(
                                    pt[:, j * 128:(j + 1) * 128],
                                    rot[:, j * 128:(j + 1) * 128], ident[:])
                            nc.vector.tensor_copy(
                                ktb[tau // 4][:, :, (tau % 4) * 128:(tau % 4 + 1) * 128],
                                pt[:, 0:256].rearrange("p (h t) -> p h t", h=2))
                            nc.vector.tensor_copy(
                                vbb[tau // 4][:, (tau % 4) * 256:(tau % 4 + 1) * 256],
                                ps[:, 256:512])
                    if 1 in ocs and tau % 4 == 3:
                        b = tau // 4
                        aotb[b] = aotp.tile([128, HQ, 512], bf16,
                                            tag="aot", name=f"aot{b}")
                        unit_queue.extend((h, b) for h in range(HQ))

                for tau in range(NT):
                    emit_A_tau(tau, (2, 0, 1))
                    if tau >= 4:
                        for _ in range(2):
                            emit_B_unit(*unit_queue.pop(0))

            # ------------- Tail: b=3 units alternating with out-proj taus ----
            with (
                tc.tile_pool(name="wop", bufs=1) as wop,
                tc.tile_pool(name="outp", bufs=4) as outp,
                tc.tile_pool(name="psC", bufs=3, space="PSUM") as psC,
            ):
                wo = wop.tile([128, HQ, D], bf16, tag="wo")
                for nck in range(8):
                    (nc.sync if nck % 2 == 0 else nc.gpsimd).dma_start(
                        wo[:, :, nck * 512:(nck + 1) * 512],
                        wo_ext[:, :, nck * 512:(nck + 1) * 512])

                def emit_C_tau(tau):
                    b = tau // 4
                    for nck in range(8):
                        o_ps = psC.tile([128, 512], f32, tag="o")
                        for h in range(HQ):
                            nc.tensor.matmul(
                                o_ps[:],
                                aotb[b][:, h, (tau % 4) * 128:(tau % 4 + 1) * 128],
                                wo[:, h, nck * 512:(nck + 1) * 512],
                                start=(h == 0), stop=(h == HQ - 1))
                        ost = outp.tile([128, 512], f32, tag="ost")
                        nc.vector.tensor_copy(ost[:], o_ps[:])
                        (nc.sync if nck % 2 == 0 else nc.gpsimd).dma_start(
                            out_ext[tau * 128:(tau + 1) * 128,
                                    nck * 512:(nck + 1) * 512],
                            ost[:])

                # alternate unit / out-proj tau, then the remaining taus
                for i in range(8):
                    emit_B_unit(*unit_queue.pop(0))
                    emit_C_tau(i)
                for tau in range(8, NT):
                    emit_C_tau(tau)

    nc.compile()
    return nc


def _rope(nc, ps, cc, ss, rot, tmp):
    """rot = ps*cc + pairswap(ps)*ss   (pairs are consecutive elements)."""
    swap = ps.rearrange("p (i two) -> p i two", two=2)[:, :, ::-1]
    nc.vector.tensor_mul(tmp.rearrange("p (i two) -> p i two", two=2), swap,
                         ss.rearrange("p (i two) -> p i two", two=2))
    nc.vector.tensor_mul(rot, ps, cc)
    nc.vector.tensor_add(rot, rot, tmp)


_NC_CACHE = None


def _get_nc():
    global _NC_CACHE
    if _NC_CACHE is None:
        _NC_CACHE = _build_nc()
    return _NC_CACHE


def _rope_tables():
    i = np.arange(HD // 2, dtype=np.float64)
    theta = np.power(10000.0, -2.0 * i / HD)
    ang = np.outer(np.arange(T, dtype=np.float64), theta)    # [T, 64]
    cos, sin = np.cos(ang), np.sin(ang)
    cc128 = np.repeat(cos, 2, axis=1)                        # [T, 128]
    ss128 = np.stack([-sin, sin], axis=-1).reshape(T, HD)    # [T, 128]
    cc = np.tile(cc128, (1, 4))                              # [T, 512]
    ss = np.tile(ss128, (1, 4))
    ropeccss = np.concatenate([cc, ss], axis=1)              # [T, 1024]
    return np.ascontiguousarray(
        ropeccss.reshape(NT, 128, 1024).transpose(1, 0, 2)).astype(BF)


def _mask01():
    # multiplicative causal mask for the diagonal 128x128 tile of a P^T strip:
    # partition tk, free tq_local; keep iff tq_local >= tk
    i = np.arange(128)
    return (i[None, :] >= i[:, None]).astype(BF)


def _prep_core_inputs(x, wq, wk, wv, wo):
    rope = _rope_tables()
    m01 = _mask01()
    ident = np.eye(128).astype(BF)
    in_maps = []
    for c in range(8):
        b, g = c // 4, c % 4
        xb = np.asarray(x[b], dtype=np.float32)
        xt = np.ascontiguousarray(
            xb.reshape(NT, 128, NC, 128).transpose(0, 3, 2, 1)).astype(BF)
        wq_g = wq[:, g * 8 * HD:(g + 1) * 8 * HD]
        wk_g = wk[:, g * 2 * HD:(g + 1) * 2 * HD]
        wv_g = wv[:, g * 2 * HD:(g + 1) * 2 * HD]
        W = np.concatenate([wq_g, wk_g, wv_g], axis=1)       # [D, 1536]
        wqkv3 = np.stack([
            np.ascontiguousarray(
                W[:, oc * 512:(oc + 1) * 512].reshape(NC, 128, 512)
                .transpose(1, 0, 2))
            for oc in range(3)], axis=0).astype(BF)          # [3, 128, NC, 512]
        wo_g = wo[g * 8 * HD:(g + 1) * 8 * HD, :]            # [1024, D]
        wo_t = np.ascontiguousarray(
            wo_g.reshape(HQ, 128, D).transpose(1, 0, 2)).astype(BF)
        in_maps.append({
            "xt": xt, "wqkv": wqkv3, "wo": wo_t,
            "rope": rope, "mask01": m01, "ident": ident,
        })
    return in_maps


def _run(inputs, trace=False, trace_kwargs=None):
    x = np.asarray(inputs["x"], dtype=np.float32)
    wq = np.asarray(inputs["wq"], dtype=np.float32)
    wk = np.asarray(inputs["wk"], dtype=np.float32)
    wv = np.asarray(inputs["wv"], dtype=np.float32)
    wo = np.asarray(inputs["wo"], dtype=np.float32)
    nc = _get_nc()
    in_maps = _prep_core_inputs(x, wq, wk, wv, wo)
    res = run_bass_kernel_spmd(nc, in_maps, core_ids=list(range(8)),
                               trace=trace, **(trace_kwargs or {}))
    out = np.zeros((B, T, D), dtype=np.float32)
    for c in range(8):
        out[c // 4] += res.results[c]["out"]
    return out, res


def kernel(**inputs):
    out, _ = _run(inputs)
    return out
